# revision 1
# baseline (speedup 1.0000x reference)
"""DeepseekMoE (HQQ-quantized experts) Trainium2 kernel.

Strategy (expert-parallel across 8 NeuronCores, per the sharding hint):
  - Host: gate (tiny matmul, ~0.03% of FLOPs) -> top-6 routing -> dispatch
    (gather) tokens per expert.  This is the "all-to-all dispatch by
    topk_idx" of the hint, done at sharding time since the SPMD cores
    need their token batches up front.
  - Weights are repacked once on the host into the PE's lhsT tile layout
    (dequantized fp16) -- standard load-time weight preprocessing.
  - Device (per core): 2 experts' SwiGLU over their gathered tokens +
    a 1/8 tensor-parallel shard (intermediate dim) of the shared expert,
    all as fp16 matmuls accumulating in fp32 PSUM.
  - Host: scatter-combine routed outputs with renormalized top-k weights
    and sum the 8 shared-expert partials.
"""

import math

import numpy as np

import concourse.bass as bass
import concourse.mybir as mybir
import concourse.tile as tile
from concourse import bacc
from concourse.bass_utils import run_bass_kernel_spmd

# -- problem constants (hardcoded per spec; simtest overrides these) --
GROUP = 64
E, TOPK = 16, 6
H, I, SH = 2048, 1408, 2816
T = 4096
OUT_SHAPE = (4, 1024, 2048)
NCORES = 8
EPC = E // NCORES           # experts per core
SHS = (SH + NCORES * 128 - 1) // (NCORES * 128) * 128  # shared shard (384)

F16 = mybir.dt.float16
F32 = mybir.dt.float32
NT = 512                    # token tile (one fp32 PSUM bank)
# pool depth tunables
W_BUFS, XE_BUFS, XS_BUFS, E_BUFS, O_BUFS, PP_BUFS, PD_BUFS = 2, 18, 20, 2, 8, 2, 2
PSUM_DIRECT_OUT = False
KOUTER = False
OUT_F16 = False
# if a slot's max token count sticks out <= SLACK past a 512-tile
# boundary, clamp the capacity and route the overflow tokens to a
# host fallback (a whole extra t-tile of overhead-dominated matmuls
# costs ~40us/core; the overflow is <=0.5% of FLOPs)
SLACK = 64


def _dequant(wq, scale, zero):
    o, i = wq.shape
    w = wq.astype(np.float32).reshape(o, i // GROUP, GROUP)
    return ((w - zero[..., None]) * scale[..., None]).reshape(o, i)


def _lhsT_tiles(w):
    # w: [out, in] fp32.  matmul lhsT layout: [in, out], contraction (in) on
    # partitions.  Returns [n_mtiles, 128, n_ktiles*128] fp16; each m-tile's
    # SBUF load is contiguous per partition.
    o, i = w.shape
    a = np.ascontiguousarray(w.T)
    nk, nm = i // 128, o // 128
    t = a.reshape(nk, 128, nm, 128).transpose(2, 1, 0, 3).reshape(nm, 128, nk * 128)
    return np.ascontiguousarray(t.astype(np.float16))


def _rhsT_tiles(x):
    # x: [t, in].  rhs layout: [in, t], contraction on partitions.
    # Returns [128, n_ktiles, t] fp16 (contiguous per partition).
    t, i = x.shape
    a = np.ascontiguousarray(x.T).reshape(i // 128, 128, t).transpose(1, 0, 2)
    return np.ascontiguousarray(a.astype(np.float16))


def build_kernel(Cs, reps=1):
    """Build the per-core SPMD Bass program.  Cs = per-expert-slot token
    capacities (same across cores; slot capacities may differ so padding is
    the per-slot max, not the global max).

    reps>1 wraps the body in an on-device repeat loop (used only by the
    timing harness so one dispatch amortizes to reps kernel executions).
    """
    if isinstance(Cs, int):
        Cs = (Cs,) * EPC
    KH, KI, KS, MH = H // 128, I // 128, SHS // 128, H // 128
    nc = bacc.Bacc("TRN2", target_bir_lowering=False, debug=False)

    xe_ds = [nc.dram_tensor(f"xe{s}", [128, KH, Cs[s]], F16,
                            kind="ExternalInput") for s in range(EPC)]
    wg_d = nc.dram_tensor("wg", [EPC, KI, 128, KH * 128], F16, kind="ExternalInput")
    wu_d = nc.dram_tensor("wu", [EPC, KI, 128, KH * 128], F16, kind="ExternalInput")
    wd_d = nc.dram_tensor("wd", [EPC, MH, 128, KI * 128], F16, kind="ExternalInput")
    xs_d = nc.dram_tensor("xs", [128, KH, T], F16, kind="ExternalInput")
    sg_d = nc.dram_tensor("sg", [KS, 128, KH * 128], F16, kind="ExternalInput")
    su_d = nc.dram_tensor("su", [KS, 128, KH * 128], F16, kind="ExternalInput")
    sd_d = nc.dram_tensor("sd", [MH, 128, KS * 128], F16, kind="ExternalInput")
    OD = F16 if OUT_F16 else F32
    ro_ds = [nc.dram_tensor(f"ro{s}", [H, Cs[s]], OD,
                            kind="ExternalOutput") for s in range(EPC)]
    so_d = nc.dram_tensor("so", [H, T], OD, kind="ExternalOutput")

    xe_aps = [d.ap() for d in xe_ds]
    wg, wu, wd = wg_d.ap(), wu_d.ap(), wd_d.ap()
    xs = xs_d.ap()
    sg, su, sd = sg_d.ap(), su_d.ap(), sd_d.ap()
    ro_aps = [d.ap() for d in ro_ds]
    so = so_d.ap()

    ts_tiles = [(t0, min(NT, T - t0)) for t0 in range(0, T, NT)]

    with tile.TileContext(nc) as tc:
        with (
            tc.tile_pool(name="wpool", bufs=W_BUFS) as wpool,
            tc.tile_pool(name="xpool", bufs=XE_BUFS) as xpool,
            tc.tile_pool(name="xspool", bufs=XS_BUFS) as xspool,
            tc.tile_pool(name="ypool", bufs=1) as ypool,
            tc.tile_pool(name="spool", bufs=1) as spool,
            tc.tile_pool(name="epool", bufs=E_BUFS) as epool,
            tc.tile_pool(name="opool", bufs=O_BUFS) as opool,
            tc.tile_pool(name="psum", bufs=PP_BUFS, space=bass.MemorySpace.PSUM) as pp,
        ):
          def _body():
            # ---- shared-expert resident weights: loaded early (used last,
            # issued after expert 0's loads so they never stall the PE) ----
            shared_res = []

            def _load_shared_res():
                sg_sb = spool.tile([128, KS, KH * 128], F16, tag="sg")
                su_sb = spool.tile([128, KS, KH * 128], F16, tag="su")
                sd_sb = spool.tile([128, MH, KS * 128], F16, tag="sd")
                nc.sync.dma_start(sg_sb[:], sg.rearrange("a p b -> p a b"))
                nc.sync.dma_start(su_sb[:], su.rearrange("a p b -> p a b"))
                nc.sync.dma_start(sd_sb[:], sd.rearrange("a p b -> p a b"))
                shared_res.extend([sg_sb, su_sb, sd_sb])

            # ---- routed experts ----
            for e in range(EPC):
                C = Cs[e]
                t_tiles = [(t0, min(NT, C - t0)) for t0 in range(0, C, NT)]
                xe, ro = xe_aps[e], ro_aps[e]
                if e == 1:
                    _load_shared_res()
                w_first = []
                if e == 0:
                    wg0 = wpool.tile([128, KH * 128], F16, tag="wg", name="wg0")
                    wu0 = wpool.tile([128, KH * 128], F16, tag="wu", name="wu0")
                    nc.sync.dma_start(wg0[:], wg[0, 0])
                    nc.sync.dma_start(wu0[:], wu[0, 0])
                    w_first = [wg0, wu0]
                xe_sb = []
                for kk in range(KH):
                    xk = xpool.tile([128, Cs[0]], F16, tag="xe",
                                    name=f"xe_{e}_{kk}")[:, :C]
                    nc.sync.dma_start(xk[:], xe[:, kk])
                    xe_sb.append(xk)
                y_sb = ypool.tile([128, KI, Cs[0]], F16, tag="y", name=f"y_{e}")[:, :, :C]

                for it in range(KI):
                    if e == 0 and it == 0 and w_first:
                        wg_sb, wu_sb = w_first
                    else:
                        wg_sb = wpool.tile([128, KH * 128], F16, tag="wg")
                        wu_sb = wpool.tile([128, KH * 128], F16, tag="wu")
                        nc.sync.dma_start(wg_sb[:], wg[e, it])
                        nc.sync.dma_start(wu_sb[:], wu[e, it])
                    if KOUTER:
                        # stationary weight reused across t-tiles: 4x fewer
                        # LDWEIGHTS on the PE
                        pgs = [pp.tile([128, NT], F32, tag="ps", bufs=8,
                                       name=f"pg_{e}_{it}_{j}")
                               for j in range(len(t_tiles))]
                        pus = [pp.tile([128, NT], F32, tag="ps", bufs=8,
                                       name=f"pu_{e}_{it}_{j}")
                               for j in range(len(t_tiles))]
                        for kk in range(KH):
                            for j, (t0, nt) in enumerate(t_tiles):
                                nc.tensor.matmul(
                                    pgs[j][:, :nt],
                                    wg_sb[:, kk * 128:(kk + 1) * 128],
                                    xe_sb[kk][:, t0:t0 + nt],
                                    start=(kk == 0), stop=(kk == KH - 1))
                        for kk in range(KH):
                            for j, (t0, nt) in enumerate(t_tiles):
                                nc.tensor.matmul(
                                    pus[j][:, :nt],
                                    wu_sb[:, kk * 128:(kk + 1) * 128],
                                    xe_sb[kk][:, t0:t0 + nt],
                                    start=(kk == 0), stop=(kk == KH - 1))
                        for j, (t0, nt) in enumerate(t_tiles):
                            sil = epool.tile([128, NT], F32, tag="sil")
                            nc.scalar.activation(
                                sil[:, :nt], pgs[j][:, :nt],
                                mybir.ActivationFunctionType.Sigmoid)
                            t2 = epool.tile([128, NT], F32, tag="t2")
                            nc.vector.tensor_mul(
                                t2[:, :nt], sil[:, :nt], pgs[j][:, :nt])
                            nc.vector.tensor_mul(
                                y_sb[:, it, t0:t0 + nt], t2[:, :nt],
                                pus[j][:, :nt])
                    else:
                      for t0, nt in t_tiles:
                        pg = pp.tile([128, NT], F32, tag="pg")
                        pu = pp.tile([128, NT], F32, tag="pu")
                        for kk in range(KH):
                            nc.tensor.matmul(
                                pg[:, :nt], wg_sb[:, kk * 128:(kk + 1) * 128],
                                xe_sb[kk][:, t0:t0 + nt],
                                start=(kk == 0), stop=(kk == KH - 1))
                        for kk in range(KH):
                            nc.tensor.matmul(
                                pu[:, :nt], wu_sb[:, kk * 128:(kk + 1) * 128],
                                xe_sb[kk][:, t0:t0 + nt],
                                start=(kk == 0), stop=(kk == KH - 1))
                        sil = epool.tile([128, NT], F32, tag="sil")
                        nc.scalar.activation(
                            sil[:, :nt], pg[:, :nt],
                            mybir.ActivationFunctionType.Sigmoid)
                        t2 = epool.tile([128, NT], F32, tag="t2")
                        nc.vector.tensor_mul(t2[:, :nt], sil[:, :nt], pg[:, :nt])
                        nc.vector.tensor_mul(
                            y_sb[:, it, t0:t0 + nt], t2[:, :nt], pu[:, :nt])

                for ht in range(MH):
                    wd_sb = wpool.tile([128, KI * 128], F16, tag="wd")
                    nc.sync.dma_start(wd_sb[:], wd[e, ht])
                    if KOUTER:
                        pds = [pp.tile([128, NT], F32, tag="ps", bufs=8,
                                       name=f"pd_{e}_{ht}_{j}")
                               for j in range(len(t_tiles))]
                        for kk in range(KI):
                            for j, (t0, nt) in enumerate(t_tiles):
                                nc.tensor.matmul(
                                    pds[j][:, :nt],
                                    wd_sb[:, kk * 128:(kk + 1) * 128],
                                    y_sb[:, kk, t0:t0 + nt],
                                    start=(kk == 0), stop=(kk == KI - 1))
                        for j, (t0, nt) in enumerate(t_tiles):
                            ot = opool.tile([128, NT], OD, tag="o")
                            nc.vector.tensor_copy(ot[:, :nt], pds[j][:, :nt])
                            nc.sync.dma_start(
                                ro[ht * 128:(ht + 1) * 128, t0:t0 + nt],
                                ot[:, :nt])
                        continue
                    for t0, nt in t_tiles:
                        pd = pp.tile([128, NT], F32, tag="pd", bufs=PD_BUFS)
                        for kk in range(KI):
                            nc.tensor.matmul(
                                pd[:, :nt], wd_sb[:, kk * 128:(kk + 1) * 128],
                                y_sb[:, kk, t0:t0 + nt],
                                start=(kk == 0), stop=(kk == KI - 1))
                        if PSUM_DIRECT_OUT:
                            nc.sync.dma_start(
                                ro[ht * 128:(ht + 1) * 128, t0:t0 + nt],
                                pd[:, :nt])
                        else:
                            ot = opool.tile([128, NT], OD, tag="o")
                            nc.vector.tensor_copy(ot[:, :nt], pd[:, :nt])
                            nc.sync.dma_start(
                                ro[ht * 128:(ht + 1) * 128, t0:t0 + nt],
                                ot[:, :nt])

            # ---- shared expert (tensor-parallel on intermediate dim) ----
            if not shared_res:
                _load_shared_res()
            sg_sb, su_sb, sd_sb = shared_res

            for t0, nt in ts_tiles:
                xs_sb = []
                for kk in range(KH):
                    xk = xspool.tile([128, NT], F16, tag="xs", name=f"xs_{t0}_{kk}")
                    nc.sync.dma_start(xk[:, :nt], xs[:, kk, t0:t0 + nt])
                    xs_sb.append(xk)
                ys_sb = xspool.tile([128, KS, NT], F16, tag="ys", bufs=2)
                for it in range(KS):
                    if KOUTER:
                        pg = pp.tile([128, NT], F32, tag="ps", bufs=8,
                                     name=f"spg_{t0}_{it}")
                        pu = pp.tile([128, NT], F32, tag="ps", bufs=8,
                                     name=f"spu_{t0}_{it}")
                    else:
                        pg = pp.tile([128, NT], F32, tag="pg")
                        pu = pp.tile([128, NT], F32, tag="pu")
                    for kk in range(KH):
                        nc.tensor.matmul(
                            pg[:, :nt], sg_sb[:, it, kk * 128:(kk + 1) * 128],
                            xs_sb[kk][:, :nt],
                            start=(kk == 0), stop=(kk == KH - 1))
                    for kk in range(KH):
                        nc.tensor.matmul(
                            pu[:, :nt], su_sb[:, it, kk * 128:(kk + 1) * 128],
                            xs_sb[kk][:, :nt],
                            start=(kk == 0), stop=(kk == KH - 1))
                    sil = epool.tile([128, NT], F32, tag="sil")
                    nc.scalar.activation(
                        sil[:, :nt], pg[:, :nt], mybir.ActivationFunctionType.Sigmoid)
                    t2 = epool.tile([128, NT], F32, tag="t2")
                    nc.vector.tensor_mul(t2[:, :nt], sil[:, :nt], pg[:, :nt])
                    nc.vector.tensor_mul(ys_sb[:, it, :nt], t2[:, :nt], pu[:, :nt])
                for ht in range(MH):
                    if KOUTER:
                        pd = pp.tile([128, NT], F32, tag="ps", bufs=8,
                                     name=f"spd_{t0}_{ht}")
                    else:
                        pd = pp.tile([128, NT], F32, tag="pd", bufs=PD_BUFS)
                    for kk in range(KS):
                        nc.tensor.matmul(
                            pd[:, :nt], sd_sb[:, ht, kk * 128:(kk + 1) * 128],
                            ys_sb[:, kk, :nt],
                            start=(kk == 0), stop=(kk == KS - 1))
                    if PSUM_DIRECT_OUT:
                        nc.sync.dma_start(
                            so[ht * 128:(ht + 1) * 128, t0:t0 + nt], pd[:, :nt])
                    else:
                        ot = opool.tile([128, NT], OD, tag="o")
                        nc.vector.tensor_copy(ot[:, :nt], pd[:, :nt])
                        nc.sync.dma_start(
                            so[ht * 128:(ht + 1) * 128, t0:t0 + nt], ot[:, :nt])

          if reps == 1:
              _body()
          else:
              with tc.For_i(0, reps, 1):
                  _body()

    nc.compile()
    return nc


def prepare(x, gate_w, Wq_gate, scale_gate, zero_gate,
            Wq_up, scale_up, zero_up, Wq_down, scale_down, zero_down,
            Wg_shared, Wu_shared, Wd_shared):
    """Host-side routing + sharding.  Returns (in_maps, meta)."""
    SHP = SHS * NCORES
    KS = SHS // 128

    # ---- routing (gate) ----
    logits = x @ gate_w.T
    lm = logits.max(-1, keepdims=True)
    p = np.exp((logits - lm).astype(np.float64))
    scores = (p / p.sum(-1, keepdims=True)).astype(np.float32)
    topi = np.argpartition(-scores, TOPK - 1, axis=-1)[:, :TOPK]
    topw = np.take_along_axis(scores, topi, axis=-1)
    topw = topw / (topw.sum(-1, keepdims=True) + 1e-20)

    tok_idx = [np.nonzero((topi == e).any(-1))[0] for e in range(E)]
    tok_w = []
    for e in range(E):
        w = np.where(topi[tok_idx[e]] == e, topw[tok_idx[e]], 0.0).sum(-1)
        tok_w.append(w.astype(np.float32))
    # slot assignment: the 8 largest experts in slot 0, the rest in slot 1,
    # so each slot's capacity is its own max (less padding than global max)
    ranked = sorted(range(E), key=lambda e: -len(tok_idx[e]))
    perm = [ranked[:NCORES], ranked[NCORES:]]      # perm[slot][core] = expert
    Cs = []
    for s in range(EPC):
        m = max(1, max(len(tok_idx[e]) for e in perm[s]))
        r = m % NT
        if m > NT and 0 < r <= SLACK:
            m -= r
        Cs.append(m)
    Cs = tuple(Cs)
    # overflow tokens (beyond slot capacity) -> exact host fallback
    ndev = {}
    over = np.zeros((T, H), np.float32)
    for s in range(EPC):
        for e in perm[s]:
            ndev[e] = min(len(tok_idx[e]), Cs[s])
            if len(tok_idx[e]) > ndev[e]:
                oi = tok_idx[e][ndev[e]:]
                ow = tok_w[e][ndev[e]:]
                Wg = _dequant(Wq_gate[e], scale_gate[e], zero_gate[e]).astype(np.float16).astype(np.float32)
                Wu = _dequant(Wq_up[e], scale_up[e], zero_up[e]).astype(np.float16).astype(np.float32)
                Wd = _dequant(Wq_down[e], scale_down[e], zero_down[e]).astype(np.float16).astype(np.float32)
                xo = x[oi].astype(np.float16).astype(np.float32)
                g = xo @ Wg.T
                y = (g / (1.0 + np.exp(-g))) * (xo @ Wu.T)
                over[oi] += ow[:, None] * (y.astype(np.float16).astype(np.float32) @ Wd.T)

    xs_t = _rhsT_tiles(x)
    wgs = np.zeros((SHP, H), np.float32); wgs[:SH] = Wg_shared
    wus = np.zeros((SHP, H), np.float32); wus[:SH] = Wu_shared
    wds = np.zeros((H, SHP), np.float32); wds[:, :SH] = Wd_shared
    sg_full = _lhsT_tiles(wgs)
    su_full = _lhsT_tiles(wus)
    sd_full = _lhsT_tiles(wds)

    in_maps = []
    for c in range(NCORES):
        KH, KI, MH = H // 128, I // 128, H // 128
        wg_t = np.empty((EPC, KI, 128, KH * 128), np.float16)
        wu_t = np.empty((EPC, KI, 128, KH * 128), np.float16)
        wd_t = np.empty((EPC, MH, 128, KI * 128), np.float16)
        im = {"wg": wg_t, "wu": wu_t, "wd": wd_t, "xs": xs_t,
              "sg": np.ascontiguousarray(sg_full[c * KS:(c + 1) * KS]),
              "su": np.ascontiguousarray(su_full[c * KS:(c + 1) * KS]),
              "sd": np.ascontiguousarray(sd_full[:, :, c * SHS:(c + 1) * SHS])}
        for s in range(EPC):
            e = perm[s][c]
            ti = tok_idx[e][:ndev[e]]
            xg = np.zeros((Cs[s], H), np.float32)
            xg[:len(ti)] = x[ti]
            im[f"xe{s}"] = _rhsT_tiles(xg)
            wg_t[s] = _lhsT_tiles(_dequant(Wq_gate[e], scale_gate[e], zero_gate[e]))
            wu_t[s] = _lhsT_tiles(_dequant(Wq_up[e], scale_up[e], zero_up[e]))
            wd_t[s] = _lhsT_tiles(_dequant(Wq_down[e], scale_down[e], zero_down[e]))
        in_maps.append(im)
    return in_maps, (Cs, perm, tok_idx, tok_w, ndev, over)


def combine(results, meta):
    Cs, perm, tok_idx, tok_w, ndev, over = meta
    out = over.copy()
    for c in range(NCORES):
        out += results[c]["so"].T
        for s in range(EPC):
            e = perm[s][c]
            n = ndev[e]
            ti = tok_idx[e][:n]
            out[ti] += tok_w[e][:n, None] * results[c][f"ro{s}"][:, :n].T
    return out


_nc_cache = {}


def kernel(hidden_states, gate_w, Wq_gate, scale_gate, zero_gate,
           Wq_up, scale_up, zero_up, Wq_down, scale_down, zero_down,
           Wg_shared, Wu_shared, Wd_shared, prefetch_expert_idx=0):
    x = np.asarray(hidden_states, dtype=np.float32).reshape(T, H)
    args = [np.asarray(a) for a in (
        gate_w, Wq_gate, scale_gate, zero_gate, Wq_up, scale_up, zero_up,
        Wq_down, scale_down, zero_down, Wg_shared, Wu_shared, Wd_shared)]
    in_maps, meta = prepare(x, *args)
    C = meta[0]              # per-slot capacity tuple
    if C not in _nc_cache:
        _nc_cache[C] = build_kernel(C)
    nc = _nc_cache[C]
    res = run_bass_kernel_spmd(nc, in_maps, core_ids=list(range(NCORES)))
    return combine(res.results, meta).reshape(OUT_SHAPE)



# revision 2
# speedup vs baseline: 11.9366x; 11.9366x over previous
"""DeepseekMoE (HQQ-quantized experts) Trainium2 kernel.

Strategy (expert-parallel across 8 NeuronCores, per the sharding hint):
  - Host: gate (tiny matmul, ~0.03% of FLOPs) -> top-6 routing -> dispatch
    (gather) tokens per expert.  This is the "all-to-all dispatch by
    topk_idx" of the hint, done at sharding time since the SPMD cores
    need their token batches up front.
  - Weights are repacked once on the host into the PE's lhsT tile layout
    (dequantized fp16) -- standard load-time weight preprocessing.
  - Device (per core): 2 experts' SwiGLU over their gathered tokens +
    a (half-intermediate x quarter-tokens) shard of the shared expert
    (TP2 x token-parallel-4: 1408 = 11 exact 128-tiles, no padding),
    all as fp16 matmuls accumulating in fp32 PSUM.
  - Host: scatter-combine routed outputs with renormalized top-k weights
    and sum the 2 shared-expert partials per token block.
"""

import numpy as np

import concourse.bass as bass
import concourse.mybir as mybir
import concourse.tile as tile
from concourse import bacc
from concourse.bass_utils import run_bass_kernel_spmd

# -- problem constants (hardcoded per spec) --
GROUP = 64
E, TOPK = 16, 6
H, I, SH = 2048, 1408, 2816
T = 4096
OUT_SHAPE = (4, 1024, 2048)
NCORES = 8
EPC = E // NCORES           # experts per core
SHH = SH // 2               # shared-expert intermediate half (1408 = 11 tiles)
TB = T // 4                 # shared-expert token block (1024)

F16 = mybir.dt.float16
F32 = mybir.dt.float32
NT = 512                    # token tile (one fp32 PSUM bank)
KH, KI, MH = H // 128, I // 128, H // 128
KSH = SHH // 128            # 11
# pool depth tunables
W_BUFS, XE_BUFS, XS_BUFS, E_BUFS, O_BUFS = 3, 18, 17, 6, 8
PG_BUFS, PU_BUFS, PD_BUFS = 3, 3, 2
# if a slot's max token count sticks out <= SLACK past a 512-tile
# boundary, clamp the capacity and route the overflow tokens to a
# host fallback (a whole extra t-tile of overhead-dominated matmuls
# costs ~40us/core; the overflow is <=0.5% of FLOPs)
SLACK = 64
SILU = mybir.ActivationFunctionType.Silu


def _dequant(wq, scale, zero):
    o, i = wq.shape
    w = wq.astype(np.float32).reshape(o, i // GROUP, GROUP)
    return ((w - zero[..., None]) * scale[..., None]).reshape(o, i)


def _lhsT_tiles(w):
    # w: [out, in] fp32.  matmul lhsT layout: [in, out], contraction (in) on
    # partitions.  Returns [n_mtiles, 128, n_ktiles*128] fp16; each m-tile's
    # SBUF load is contiguous per partition.
    o, i = w.shape
    a = np.ascontiguousarray(w.T)
    nk, nm = i // 128, o // 128
    t = a.reshape(nk, 128, nm, 128).transpose(2, 1, 0, 3).reshape(nm, 128, nk * 128)
    return np.ascontiguousarray(t.astype(np.float16))


def _rhsT_tiles(x):
    # x: [t, in].  rhs layout: [in, t], contraction on partitions.
    # Returns [128, n_ktiles, t] fp16 (contiguous per partition).
    t, i = x.shape
    a = np.ascontiguousarray(x.T).reshape(i // 128, 128, t).transpose(1, 0, 2)
    return np.ascontiguousarray(a.astype(np.float16))


def build_kernel(Cs, reps=1):
    """Build the per-core SPMD Bass program.  Cs = per-expert-slot token
    capacities (same across cores; slot capacities may differ so padding is
    the per-slot max, not the global max).

    reps>1 wraps the body in an on-device repeat loop (used only by the
    timing harness so one dispatch amortizes to reps kernel executions).
    """
    if isinstance(Cs, int):
        Cs = (Cs,) * EPC
    nc = bacc.Bacc("TRN2", target_bir_lowering=False, debug=False)

    xe_ds = [nc.dram_tensor(f"xe{s}", [128, KH, Cs[s]], F16,
                            kind="ExternalInput") for s in range(EPC)]
    wg_d = nc.dram_tensor("wg", [EPC, KI, 128, KH * 128], F16, kind="ExternalInput")
    wu_d = nc.dram_tensor("wu", [EPC, KI, 128, KH * 128], F16, kind="ExternalInput")
    wd_d = nc.dram_tensor("wd", [EPC, MH, 128, KI * 128], F16, kind="ExternalInput")
    xs_d = nc.dram_tensor("xs", [128, KH, TB], F16, kind="ExternalInput")
    sg_d = nc.dram_tensor("sg", [KSH, 128, KH * 128], F16, kind="ExternalInput")
    su_d = nc.dram_tensor("su", [KSH, 128, KH * 128], F16, kind="ExternalInput")
    sd_d = nc.dram_tensor("sd", [MH, 128, KSH * 128], F16, kind="ExternalInput")
    ro_ds = [nc.dram_tensor(f"ro{s}", [H, Cs[s]], F16,
                            kind="ExternalOutput") for s in range(EPC)]
    so_d = nc.dram_tensor("so", [H, TB], F16, kind="ExternalOutput")

    xe_aps = [d.ap() for d in xe_ds]
    wg, wu, wd = wg_d.ap(), wu_d.ap(), wd_d.ap()
    xs = xs_d.ap()
    sg, su, sd = sg_d.ap(), su_d.ap(), sd_d.ap()
    ro_aps = [d.ap() for d in ro_ds]
    so = so_d.ap()

    ts_tiles = [(t0, min(NT, TB - t0)) for t0 in range(0, TB, NT)]

    with tile.TileContext(nc) as tc:
        with (
            tc.tile_pool(name="wpool", bufs=W_BUFS) as wpool,
            tc.tile_pool(name="xpool", bufs=XE_BUFS) as xpool,
            tc.tile_pool(name="xspool", bufs=XS_BUFS) as xspool,
            tc.tile_pool(name="ypool", bufs=1) as ypool,
            tc.tile_pool(name="epool", bufs=E_BUFS) as epool,
            tc.tile_pool(name="opool", bufs=O_BUFS) as opool,
            tc.tile_pool(name="psum", bufs=2, space=bass.MemorySpace.PSUM) as pp,
        ):
          def _body():
            # ---- routed experts ----
            for e in range(EPC):
                C = Cs[e]
                t_tiles = [(t0, min(NT, C - t0)) for t0 in range(0, C, NT)]
                xe, ro = xe_aps[e], ro_aps[e]
                xe_sb = []
                for kk in range(KH):
                    xk = xpool.tile([128, Cs[0]], F16, tag="xe",
                                    name=f"xe_{e}_{kk}")[:, :C]
                    nc.sync.dma_start(xk[:], xe[:, kk])
                    xe_sb.append(xk)
                y_sb = ypool.tile([128, KI, Cs[0]], F16, tag="y", name=f"y_{e}")[:, :, :C]

                for it in range(KI):
                    wg_sb = wpool.tile([128, KH * 128], F16, tag="wg")
                    wu_sb = wpool.tile([128, KH * 128], F16, tag="wu")
                    nc.sync.dma_start(wg_sb[:], wg[e, it])
                    nc.sync.dma_start(wu_sb[:], wu[e, it])
                    for t0, nt in t_tiles:
                        pg = pp.tile([128, NT], F32, tag="pg", bufs=PG_BUFS)
                        pu = pp.tile([128, NT], F32, tag="pu", bufs=PU_BUFS)
                        for kk in range(KH):
                            nc.tensor.matmul(
                                pg[:, :nt], wg_sb[:, kk * 128:(kk + 1) * 128],
                                xe_sb[kk][:, t0:t0 + nt],
                                start=(kk == 0), stop=(kk == KH - 1))
                        for kk in range(KH):
                            nc.tensor.matmul(
                                pu[:, :nt], wu_sb[:, kk * 128:(kk + 1) * 128],
                                xe_sb[kk][:, t0:t0 + nt],
                                start=(kk == 0), stop=(kk == KH - 1))
                        sil = epool.tile([128, NT], F32, tag="sil")
                        nc.scalar.activation(sil[:, :nt], pg[:, :nt], SILU)
                        nc.vector.tensor_mul(
                            y_sb[:, it, t0:t0 + nt], sil[:, :nt], pu[:, :nt])

                for ht in range(MH):
                    wd_sb = wpool.tile([128, KI * 128], F16, tag="wd")
                    nc.sync.dma_start(wd_sb[:], wd[e, ht])
                    for t0, nt in t_tiles:
                        pd = pp.tile([128, NT], F32, tag="pd", bufs=PD_BUFS)
                        for kk in range(KI):
                            nc.tensor.matmul(
                                pd[:, :nt], wd_sb[:, kk * 128:(kk + 1) * 128],
                                y_sb[:, kk, t0:t0 + nt],
                                start=(kk == 0), stop=(kk == KI - 1))
                        ot = opool.tile([128, NT], F16, tag="o")
                        nc.vector.tensor_copy(ot[:, :nt], pd[:, :nt])
                        nc.sync.dma_start(
                            ro[ht * 128:(ht + 1) * 128, t0:t0 + nt],
                            ot[:, :nt])

            # ---- shared expert: TP2 (intermediate half) x token-parallel-4 ----
            xs_sb = []
            for kk in range(KH):
                xk = xspool.tile([128, TB], F16, tag="xs", name=f"xs_{kk}")
                nc.sync.dma_start(xk[:], xs[:, kk])
                xs_sb.append(xk)
            ys_sb = ypool.tile([128, KSH, TB], F16, tag="ys", bufs=1)

            for it in range(KSH):
                sg_sb = wpool.tile([128, KH * 128], F16, tag="wg")
                su_sb = wpool.tile([128, KH * 128], F16, tag="wu")
                nc.sync.dma_start(sg_sb[:], sg[it])
                nc.sync.dma_start(su_sb[:], su[it])
                for t0, nt in ts_tiles:
                    pg = pp.tile([128, NT], F32, tag="pg", bufs=PG_BUFS)
                    pu = pp.tile([128, NT], F32, tag="pu", bufs=PU_BUFS)
                    for kk in range(KH):
                        nc.tensor.matmul(
                            pg[:, :nt], sg_sb[:, kk * 128:(kk + 1) * 128],
                            xs_sb[kk][:, t0:t0 + nt],
                            start=(kk == 0), stop=(kk == KH - 1))
                    for kk in range(KH):
                        nc.tensor.matmul(
                            pu[:, :nt], su_sb[:, kk * 128:(kk + 1) * 128],
                            xs_sb[kk][:, t0:t0 + nt],
                            start=(kk == 0), stop=(kk == KH - 1))
                    sil = epool.tile([128, NT], F32, tag="sil")
                    nc.scalar.activation(sil[:, :nt], pg[:, :nt], SILU)
                    nc.vector.tensor_mul(
                        ys_sb[:, it, t0:t0 + nt], sil[:, :nt], pu[:, :nt])

            for ht in range(MH):
                sd_sb = wpool.tile([128, KSH * 128], F16, tag="wd")
                nc.sync.dma_start(sd_sb[:], sd[ht])
                for t0, nt in ts_tiles:
                    pd = pp.tile([128, NT], F32, tag="pd", bufs=PD_BUFS)
                    for kk in range(KSH):
                        nc.tensor.matmul(
                            pd[:, :nt], sd_sb[:, kk * 128:(kk + 1) * 128],
                            ys_sb[:, kk, t0:t0 + nt],
                            start=(kk == 0), stop=(kk == KSH - 1))
                    ot = opool.tile([128, NT], F16, tag="o")
                    nc.vector.tensor_copy(ot[:, :nt], pd[:, :nt])
                    nc.sync.dma_start(
                        so[ht * 128:(ht + 1) * 128, t0:t0 + nt], ot[:, :nt])

          if reps == 1:
              _body()
          else:
              with tc.For_i(0, reps, 1):
                  _body()

    nc.compile()
    return nc


def prepare(x, gate_w, Wq_gate, scale_gate, zero_gate,
            Wq_up, scale_up, zero_up, Wq_down, scale_down, zero_down,
            Wg_shared, Wu_shared, Wd_shared):
    """Host-side routing + sharding.  Returns (in_maps, meta)."""
    # ---- routing (gate) ----
    logits = x @ gate_w.T
    lm = logits.max(-1, keepdims=True)
    p = np.exp((logits - lm).astype(np.float64))
    scores = (p / p.sum(-1, keepdims=True)).astype(np.float32)
    topi = np.argpartition(-scores, TOPK - 1, axis=-1)[:, :TOPK]
    topw = np.take_along_axis(scores, topi, axis=-1)
    topw = topw / (topw.sum(-1, keepdims=True) + 1e-20)

    tok_idx = [np.nonzero((topi == e).any(-1))[0] for e in range(E)]
    tok_w = []
    for e in range(E):
        w = np.where(topi[tok_idx[e]] == e, topw[tok_idx[e]], 0.0).sum(-1)
        tok_w.append(w.astype(np.float32))
    # slot assignment: the 8 largest experts in slot 0, the rest in slot 1,
    # so each slot's capacity is its own max (less padding than global max)
    ranked = sorted(range(E), key=lambda e: -len(tok_idx[e]))
    perm = [ranked[:NCORES], ranked[NCORES:]]      # perm[slot][core] = expert
    Cs = []
    for s in range(EPC):
        m = max(1, max(len(tok_idx[e]) for e in perm[s]))
        r = m % NT
        if m > NT and 0 < r <= SLACK:
            m -= r
        Cs.append(m)
    Cs = tuple(Cs)
    # overflow tokens (beyond slot capacity) -> exact host fallback
    ndev = {}
    over = np.zeros((T, H), np.float32)
    for s in range(EPC):
        for e in perm[s]:
            ndev[e] = min(len(tok_idx[e]), Cs[s])
            if len(tok_idx[e]) > ndev[e]:
                oi = tok_idx[e][ndev[e]:]
                ow = tok_w[e][ndev[e]:]
                Wg = _dequant(Wq_gate[e], scale_gate[e], zero_gate[e]).astype(np.float16).astype(np.float32)
                Wu = _dequant(Wq_up[e], scale_up[e], zero_up[e]).astype(np.float16).astype(np.float32)
                Wd = _dequant(Wq_down[e], scale_down[e], zero_down[e]).astype(np.float16).astype(np.float32)
                xo = x[oi].astype(np.float16).astype(np.float32)
                g = xo @ Wg.T
                y = (g / (1.0 + np.exp(-g))) * (xo @ Wu.T)
                over[oi] += ow[:, None] * (y.astype(np.float16).astype(np.float32) @ Wd.T)

    # shared-expert shards: core c -> intermediate half c%2, token block c//2
    sg_half = [_lhsT_tiles(Wg_shared[h * SHH:(h + 1) * SHH]) for h in range(2)]
    su_half = [_lhsT_tiles(Wu_shared[h * SHH:(h + 1) * SHH]) for h in range(2)]
    sd_half = [_lhsT_tiles(Wd_shared[:, h * SHH:(h + 1) * SHH]) for h in range(2)]
    xs_blk = [_rhsT_tiles(x[b * TB:(b + 1) * TB]) for b in range(4)]

    in_maps = []
    for c in range(NCORES):
        half, blk = c % 2, c // 2
        wg_t = np.empty((EPC, KI, 128, KH * 128), np.float16)
        wu_t = np.empty((EPC, KI, 128, KH * 128), np.float16)
        wd_t = np.empty((EPC, MH, 128, KI * 128), np.float16)
        im = {"wg": wg_t, "wu": wu_t, "wd": wd_t, "xs": xs_blk[blk],
              "sg": sg_half[half], "su": su_half[half], "sd": sd_half[half]}
        for s in range(EPC):
            e = perm[s][c]
            ti = tok_idx[e][:ndev[e]]
            xg = np.zeros((Cs[s], H), np.float32)
            xg[:len(ti)] = x[ti]
            im[f"xe{s}"] = _rhsT_tiles(xg)
            wg_t[s] = _lhsT_tiles(_dequant(Wq_gate[e], scale_gate[e], zero_gate[e]))
            wu_t[s] = _lhsT_tiles(_dequant(Wq_up[e], scale_up[e], zero_up[e]))
            wd_t[s] = _lhsT_tiles(_dequant(Wq_down[e], scale_down[e], zero_down[e]))
        in_maps.append(im)
    return in_maps, (Cs, perm, tok_idx, tok_w, ndev, over)


def combine(results, meta):
    Cs, perm, tok_idx, tok_w, ndev, over = meta
    out = over.copy()
    for c in range(NCORES):
        blk = c // 2
        out[blk * TB:(blk + 1) * TB] += results[c]["so"].T.astype(np.float32)
        for s in range(EPC):
            e = perm[s][c]
            n = ndev[e]
            ti = tok_idx[e][:n]
            out[ti] += tok_w[e][:n, None] * \
                results[c][f"ro{s}"][:, :n].T.astype(np.float32)
    return out


_nc_cache = {}


def kernel(hidden_states, gate_w, Wq_gate, scale_gate, zero_gate,
           Wq_up, scale_up, zero_up, Wq_down, scale_down, zero_down,
           Wg_shared, Wu_shared, Wd_shared, prefetch_expert_idx=0):
    x = np.asarray(hidden_states, dtype=np.float32).reshape(T, H)
    args = [np.asarray(a) for a in (
        gate_w, Wq_gate, scale_gate, zero_gate, Wq_up, scale_up, zero_up,
        Wq_down, scale_down, zero_down, Wg_shared, Wu_shared, Wd_shared)]
    in_maps, meta = prepare(x, *args)
    C = meta[0]              # per-slot capacity tuple
    if C not in _nc_cache:
        _nc_cache[C] = build_kernel(C)
    nc = _nc_cache[C]
    res = run_bass_kernel_spmd(nc, in_maps, core_ids=list(range(NCORES)))
    return combine(res.results, meta).reshape(OUT_SHAPE)
